# revision 18
# baseline (speedup 1.0000x reference)
"""Multi-head causal attention on 8 Trainium2 NeuronCores - v2.

Tensor-parallel over heads (2/core). Single interleaved PE instruction
stream: QKV projection chunk waves, attention units and out-projection
units are emitted into one dense sequence so the PE never idles, HAM
stays warm, and ACT/DVE softmax work hides under matmuls.

Data is bf16 (inputs, Q/K/V, exp(S), attention out, W_o, y partials);
PSUM accumulation stays f32. Softmax denominator is computed on the PE:
db[128,512] += ones128.T @ e per entry (clean pairs pre-summed on DVE
at 2x bf16 rate), so no partition-reduce chains and no [1,512] ops;
1/d via reciprocal_approx_fast on [128,512], one DVE multiply
(o_ps PSUM operand) produces the normalized attention out.

PSUM budget (8 banks): qk 2 + v 1 + s 2 + o 1 + db 1 + y 1.
"""
import sys
if '/opt/trn_rl_repo' not in sys.path:
    sys.path.insert(0, '/opt/trn_rl_repo')

import numpy as np

B, S, D = 2, 2048, 2048
H, DK = 16, 128
NCORES = 8
HPC = H // NCORES            # heads per core
T = B * S                    # tokens
QB = 512                     # q-block width
NKT = S // 128               # k tiles per batch (16)
NQB = S // QB                # q blocks per batch (4)
NCH = T // QB                # token chunks (8)
NDT = D // 128               # d_model tiles (16)

_cache = {}


def _analyze_mask(m2):
    """m2: [S, S] bool. Returns blocks[qb] = list of entries
    (j, q0, m0, m1) ascending j: q0 first valid col (block-local),
    m0..m1 mask-multiply range (None if fully valid from q0)."""
    blocks = []
    for qb in range(NQB):
        entries = []
        for j in range(NKT):
            blk = m2[qb * QB:(qb + 1) * QB, j * 128:(j + 1) * 128]
            col_any = blk.any(axis=1)
            if not col_any.any():
                continue
            col_all = blk.all(axis=1)
            q0 = int(np.argmax(col_any))
            rev = col_all[::-1]
            run = int(np.argmin(rev)) if not rev.all() else QB
            q1 = QB - run
            if q1 <= q0:
                entries.append((j, q0, None, None))
            else:
                entries.append((j, q0, q0, q1))
        blocks.append(entries)
    return blocks


def _build(mask_bool):
    from contextlib import ExitStack
    import concourse.bass as bass
    import concourse.tile as tile
    from concourse import bacc, mybir

    f32 = mybir.dt.float32
    f32r = mybir.dt.float32r
    bf16 = mybir.dt.bfloat16
    EXP = mybir.ActivationFunctionType.Exp
    scale = 1.0 / np.sqrt(DK)

    m2 = mask_bool
    blocks = _analyze_mask(m2)
    for ents in blocks:
        assert ents and ents[0][1] == min(e[1] for e in ents), \
            "first entry must cover the widest q range"

    nc = bacc.Bacc("TRN2", target_bir_lowering=False, debug=False)
    # xt packed host-side as [128, (c*16+kd)*512 + t] so one DMA per chunk
    xt_d = nc.dram_tensor("xt", [128, NCH * NDT * 512], bf16,
                          kind="ExternalInput")
    # wqk packed as [128, kd*512 + e*128 + col]; wv as [128, kd*256 + col]
    wqk_d = nc.dram_tensor("wqk", [128, NDT * 512], bf16,
                           kind="ExternalInput")
    wv_d = nc.dram_tensor("wv", [128, NDT * 256], bf16,
                          kind="ExternalInput")
    wo_d = nc.dram_tensor("wo", [2 * 128, D], bf16, kind="ExternalInput")
    mt_d = nc.dram_tensor("mt", [S, S], bf16, kind="ExternalInput")
    y_d = nc.dram_tensor("y", [T, D], bf16, kind="ExternalOutput")
    import os as _os
    dump = bool(_os.environ.get("KERNEL_DUMP"))
    if dump:
        qk_dump = nc.dram_tensor("qk_dump", [512, T], bf16,
                                 kind="ExternalOutput")
        v_dump = nc.dram_tensor("v_dump", [128, (T // 128) * 256], bf16,
                                kind="ExternalOutput")
        at_dump = nc.dram_tensor("at_dump", [256, T], bf16,
                                 kind="ExternalOutput")
        d_dump = nc.dram_tensor("d_dump", [128, 512], f32,
                                kind="ExternalOutput")

    with tile.TileContext(nc) as tc:
        with ExitStack() as stack:
            stack.enter_context(
                nc.allow_low_precision(reason="bf16 kernel"))
            qkt_pool = stack.enter_context(tc.tile_pool(name="qkt", bufs=1))
            v_pool = stack.enter_context(tc.tile_pool(name="vsb", bufs=1))
            att_pool = stack.enter_context(tc.tile_pool(name="att", bufs=1))
            cst_pool = stack.enter_context(tc.tile_pool(name="cst", bufs=1))
            w_pool = stack.enter_context(tc.tile_pool(name="wts", bufs=1))
            xt_pool = stack.enter_context(tc.tile_pool(name="xt", bufs=3))
            e_pool = stack.enter_context(tc.tile_pool(name="e", bufs=8))
            es_pool = stack.enter_context(tc.tile_pool(name="es", bufs=6))
            acc_pool = stack.enter_context(tc.tile_pool(name="acc", bufs=3))
            rcp_pool = stack.enter_context(tc.tile_pool(name="rcp", bufs=2))
            msk_pool = stack.enter_context(tc.tile_pool(name="msk", bufs=1))
            ysb_pool = stack.enter_context(tc.tile_pool(name="ysb", bufs=4))

            s_ps_pool = stack.enter_context(
                tc.tile_pool(name="ps_s", bufs=1, space="PSUM"))
            o_ps_pool = stack.enter_context(
                tc.tile_pool(name="ps_o", bufs=1, space="PSUM"))
            db_ps_pool = stack.enter_context(
                tc.tile_pool(name="ps_db", bufs=1, space="PSUM"))
            y_ps_pool = stack.enter_context(
                tc.tile_pool(name="ps_y", bufs=1, space="PSUM"))
            # phase-1 pools opened last so they can close before the drain
            p1 = ExitStack()
            qk_ps_pool = p1.enter_context(
                tc.tile_pool(name="ps_qk", bufs=1, space="PSUM"))
            v_ps_pool = p1.enter_context(
                tc.tile_pool(name="ps_v", bufs=2, space="PSUM"))
            # drain-phase extra pools (opened after phase-1 pools close)
            s_pools = [s_ps_pool]
            y_pools = [y_ps_pool]

            # ------------ persistent SBUF ------------
            qt_sb = [qkt_pool.tile([128, T], bf16, tag=f"qt{h}", name=f"qt{h}")
                     for h in range(HPC)]
            kt_sb = [qkt_pool.tile([128, T], bf16, tag=f"kt{h}", name=f"kt{h}")
                     for h in range(HPC)]
            v_sb = v_pool.tile([128, (T // 128) * 256], bf16, tag="v")
            at_sb = [att_pool.tile([128, T], bf16, tag=f"at{h}", name=f"at{h}")
                     for h in range(HPC)]

            ones_f = cst_pool.tile([128, 128], f32, tag="ones_f")
            nc.vector.memset(ones_f[:], 1.0)
            ones_fr = cst_pool.tile([128, 128], f32r, tag="ones_fr")
            nc.scalar.copy(ones_fr[:], ones_f[:])
            # warm up the exp table early (ACT_TABLE_LOAD ~2.7us)
            exp_warm = cst_pool.tile([1, 1], f32, tag="expw")
            nc.scalar.activation(exp_warm[:], ones_f[0:1, 0:1], EXP)

            # ------------ weights: few large DMAs ------
            wqk_sb = w_pool.tile([128, NDT * 512], bf16, tag="wqk")
            wv_sb = w_pool.tile([128, NDT * 256], bf16, tag="wv")
            wo_sb = []
            for h in range(HPC):
                wt = w_pool.tile([128, D], bf16, tag=f"wo{h}", name=f"wo{h}")
                wo_sb.append(wt)

            # xt tiles: one [128, 16*512] per chunk (cols kd*512 + t)
            xt_tiles = {}

            def xt_tile(c, split=1):
                t = xt_tiles.get(c)
                if t is None:
                    t = xt_pool.tile([128, NDT * 512], bf16, tag="xt")
                    w = NDT * 512 // split
                    for i in range(split):
                        # gpsimd queue: it is otherwise idle, so xt DMA
                        # issue runs parallel to sync-queue traffic
                        nc.sync.dma_start(
                            t[:, i * w:(i + 1) * w],
                            xt_d.ap()[:, c * NDT * 512 + i * w:
                                      c * NDT * 512 + (i + 1) * w])
                    xt_tiles[c] = t
                return t

            # mask tile cache keyed by content
            mask_tiles = {}

            def mask_tile(j, qb, m0, m1):
                key = m2[qb * QB + m0:qb * QB + m1,
                         j * 128:(j + 1) * 128].tobytes()
                t = mask_tiles.get(key)
                if t is None:
                    t = msk_pool.tile([128, QB], bf16,
                                      name=f"mask{len(mask_tiles)}",
                                      tag=f"m{len(mask_tiles)}")
                    nc.sync.dma_start(
                        t[:, 0:m1 - m0],
                        mt_d.ap()[j * 128:(j + 1) * 128,
                                  qb * QB + m0:qb * QB + m1])
                    mask_tiles[key] = t
                return t

            # ---------------- attention stream ----------------
            class Stream:
                """One (b, h, qb): units alternate S/exp and d/PV."""

                def __init__(self, b, h, qb):
                    self.b, self.h, self.qb = b, h, qb
                    self.tb = b * S
                    ents = blocks[qb]
                    self.groups = [(ents[i:i + 2], i)
                                   for i in range(0, len(ents), 2)]
                    assert ents[0][1] == 0, "first entry must cover q0=0"
                    self.ne = len(ents)
                    self.qcol = self.tb + qb * QB
                    self.o_ps = None
                    self.es_acc = None
                    self.gi = 0
                    self.pends = []
                    self.done_units = False
                    self.inline_proj = False

                def unit_s(self, grp):
                    """S matmuls + exp + mask + denominator chain."""
                    h, tb = self.h, self.tb
                    q0g = grp[0][1]
                    n = len(grp)
                    sp = s_pools[sched['sc'] % len(s_pools)]
                    sched['sc'] += 1
                    s_t = sp.tile([128, 2 * QB], f32, tag="s",
                                  name="sps")
                    for idx, (j, q0, m0, m1) in enumerate(grp):
                        nc.tensor.matmul(
                            s_t[:, idx * QB + q0:(idx + 1) * QB],
                            kt_sb[h][:, tb + j * 128:tb + (j + 1) * 128],
                            qt_sb[h][:, self.qcol + q0:self.qcol + QB],
                            start=True, stop=True)
                    e_t = e_pool.tile([128, 2 * QB], bf16, tag="e",
                                      name="esb")
                    # one exp over [q0g, n*QB): start=True zeroed the
                    # whole bank, so unwritten score cols read exp(0)=1
                    # and are zeroed by the extended mask below
                    nc.scalar.activation(e_t[:, q0g:n * QB],
                                         s_t[:, q0g:n * QB], EXP,
                                         scale=scale)
                    for idx, (j, q0, m0, m1) in enumerate(grp):
                        hi = m1 if m1 is not None else \
                            (q0 if q0 > q0g else None)
                        if hi is not None and hi > q0g:
                            mt = mask_tile(j, self.qb, q0g, hi)
                            nc.vector.tensor_mul(
                                e_t[:, idx * QB + q0g:idx * QB + hi],
                                e_t[:, idx * QB + q0g:idx * QB + hi],
                                mt[:, 0:hi - q0g])
                    # denominator chain: es = e0 + e1, acc += es (f32)
                    if n == 2:
                        es_t = es_pool.tile([128, QB], bf16, tag="es",
                                            name="essb")
                        nc.vector.tensor_add(es_t[:, q0g:QB],
                                             e_t[:, q0g:QB],
                                             e_t[:, QB + q0g:2 * QB])
                        es_ap = es_t[:, q0g:QB]
                    else:
                        es_ap = e_t[:, q0g:QB]
                    if self.es_acc is None:
                        self.es_acc = acc_pool.tile([128, QB], f32r,
                                                    tag="acc", name="acc")
                        nc.vector.tensor_copy(self.es_acc[:, q0g:QB],
                                              es_ap)
                    else:
                        nc.vector.tensor_add(self.es_acc[:, q0g:QB],
                                             self.es_acc[:, q0g:QB],
                                             es_ap)
                    return (grp, e_t)

                def unit_pv(self, pend, g0):
                    """PV matmuls for a completed group."""
                    grp, e_t = pend
                    b, h = self.b, self.h
                    if self.o_ps is None:
                        self.o_ps = o_ps_pool.tile([128, QB], f32,
                                                   tag="o", name="ops")
                    for idx, (j, q0, m0, m1) in enumerate(grp):
                        gi = g0 + idx
                        nc.tensor.matmul(
                            self.o_ps[:, q0:QB],
                            v_sb[:, (b * NKT + j) * 256 + h * 128:
                                 (b * NKT + j) * 256 + (h + 1) * 128],
                            e_t[:, idx * QB + q0:(idx + 1) * QB],
                            start=(gi == 0), stop=(gi == self.ne - 1))

                def unit_tail(self):
                    if (self.h == HPC - 1
                            and (self.b, self.qb) == last_bqb):
                        # final stream: pipeline the tail per token-tile
                        # with inline projection (nothing else is left to
                        # hide the serial d->recip->mul chain behind)
                        db_ps = db_ps_pool.tile([128, QB], f32,
                                                tag="db", name="dbps")
                        rcp = rcp_pool.tile([128, QB], f32, tag="rcp",
                                            name="rcp")
                        for q4 in range(4):
                            sl = slice(q4 * 128, (q4 + 1) * 128)
                            nc.tensor.matmul(
                                db_ps[:, sl], ones_fr[:],
                                self.es_acc[:, sl],
                                start=q4 == 0, stop=q4 == 3)
                            nc.vector.reciprocal_approx_fast(
                                rcp[:, sl], db_ps[:, sl])
                            nc.vector.tensor_mul(
                                at_sb[self.h][:, self.qcol + q4 * 128:
                                              self.qcol + (q4 + 1) * 128],
                                self.o_ps[:, sl], rcp[:, sl])
                            for ch in range(4):
                                do_proj(self.b, self.qb * 4 + q4, ch)
                        self.inline_proj = True
                        return
                    db_ps = db_ps_pool.tile([128, QB], f32,
                                            tag="db", name="dbps")
                    nc.tensor.matmul(db_ps[:], ones_fr[:],
                                     self.es_acc[:], start=True,
                                     stop=True)
                    if dump and self.b == 0 and self.h == 0 and self.qb == 0:
                        dtmp = rcp_pool.tile([128, QB], f32, tag="dtmp",
                                             name="dtmp")
                        nc.vector.tensor_copy(dtmp[:], db_ps[:])
                        nc.sync.dma_start(d_dump.ap()[:, :], dtmp[:])
                    rcp = rcp_pool.tile([128, QB], f32, tag="rcp",
                                        name="rcp")
                    nc.vector.reciprocal_approx_fast(rcp[:], db_ps[:])
                    nc.vector.tensor_mul(
                        at_sb[self.h][:, self.qcol:self.qcol + QB],
                        self.o_ps[:], rcp[:])

                def step(self, s_only=False):
                    """Emit one unit. s_only: S/exp lookahead without PV
                    (safe while an older stream still owns the o bank)."""
                    if s_only:
                        if (self.gi < len(self.groups)
                                and len(self.pends) < 2):
                            grp, g0 = self.groups[self.gi]
                            self.pends.append((self.unit_s(grp), g0))
                            self.gi += 1
                            return True
                        return False
                    if self.gi < len(self.groups):
                        grp, g0 = self.groups[self.gi]
                        nxt = self.unit_s(grp)
                        if self.pends:
                            pend, pg = self.pends.pop(0)
                            self.unit_pv(pend, pg)
                        self.pends.append((nxt, g0))
                        self.gi += 1
                        return True
                    if self.pends:
                        pend, pg = self.pends.pop(0)
                        self.unit_pv(pend, pg)
                        return True
                    if not self.done_units:
                        self.unit_tail()
                        self.done_units = True
                        return True
                    return False

            # ---------------- projection units ----------------
            proj_queue = []   # (b, tt, ch)

            # chunk emission order interleaves batches so attention work
            # is available from the second chunk onward
            chunk_order = [0, 4, 1, 5, 2, 6, 3, 7]
            ysb_tiles = {}
            last_bqb = (chunk_order[-1] // 4, chunk_order[-1] % 4)

            def emit_proj_unit():
                do_proj(*proj_queue.pop(0))

            def do_proj(b, tt, ch):
                trow = b * S + tt * 128
                yp = y_pools[sched['yc'] % len(y_pools)]
                sched['yc'] += 1
                y_ps = yp.tile([128, 512], f32, tag="y", name="yps")
                for hh in range(HPC):
                    nc.tensor.matmul(
                        y_ps[:],
                        at_sb[hh][:, trow:trow + 128],
                        wo_sb[hh][:, ch * 512:(ch + 1) * 512],
                        start=(hh == 0), stop=(hh == HPC - 1))
                y_sb = ysb_tiles.get((b, tt))
                if y_sb is None:
                    y_sb = ysb_pool.tile([128, D], bf16, tag="ysb",
                                         name="ysb")
                    ysb_tiles[(b, tt)] = y_sb
                if (tt + ch) % 2 == 0:
                    nc.scalar.copy(y_sb[:, ch * 512:(ch + 1) * 512],
                                   y_ps[:])
                else:
                    nc.vector.tensor_copy(y_sb[:, ch * 512:(ch + 1) * 512],
                                          y_ps[:])
                last_qb = (b, tt // 4) == (last_bqb[0], last_bqb[1])
                if last_qb:
                    # final q-block: per-column-block DMA so the output
                    # transfers start before the kernel tail
                    nc.sync.dma_start(
                        y_d.ap()[trow:trow + 128,
                                 ch * 512:(ch + 1) * 512],
                        y_sb[:, ch * 512:(ch + 1) * 512])
                    if ch == 3:
                        del ysb_tiles[(b, tt)]
                elif ch == 3:
                    # all four column blocks done: one DMA per token row
                    nc.sync.dma_start(
                        y_d.ap()[trow:trow + 128, :], y_sb[:])
                    del ysb_tiles[(b, tt)]

            # ---------------- scheduler ----------------
            stream_list = []    # in ready order with chunk gates
            for qb in range(NQB):
                for b in range(B):
                    for h in range(HPC):
                        stream_list.append((4 * b + qb, Stream(b, h, qb)))
            sched = {'si': 0, 'tog': False, 'sc': 0, 'yc': 0, 'rr': 0}
            chunks_done = set()
            qk_done = set()     # chunk's q/k ready (before its v waves)

            def cur_stream():
                si = sched['si']
                if si >= len(stream_list):
                    return None
                gate, st = stream_list[si]
                if gate not in chunks_done:
                    return None
                return st

            def stream_done(st):
                return (st.done_units and not st.pends
                        and st.gi >= len(st.groups))

            def step_stream():
                # lead stream runs normally; the next stream may prefetch
                # S/exp units (no PV) so ACT stays fed during tails
                base = sched['si']
                rr = sched['rr']
                sched['rr'] = 1 - rr
                cands = [(base, False), (base + 1, True)]
                if rr:
                    cands.reverse()
                for si, s_only in cands:
                    if si >= len(stream_list):
                        continue
                    gate, st = stream_list[si]
                    if gate not in qk_done or stream_done(st):
                        continue
                    if gate not in chunks_done:
                        s_only = True   # v not ready: S/exp units only
                    if not st.step(s_only=s_only):
                        continue
                    while sched['si'] < len(stream_list):
                        g2, s2 = stream_list[sched['si']]
                        if not stream_done(s2):
                            break
                        if s2.h == HPC - 1 and not s2.inline_proj:
                            for t4 in range(4):
                                for ch in range(4):
                                    proj_queue.append(
                                        (s2.b, s2.qb * 4 + t4, ch))
                        sched['si'] += 1
                    return True
                return False

            def fill_slot():
                # alternate stream units and projection units; drain the
                # proj queue faster when it backs up
                tog = sched['tog']
                sched['tog'] = not tog
                if tog and proj_queue:
                    emit_proj_unit()
                    return True
                if step_stream():
                    return True
                if proj_queue:
                    emit_proj_unit()
                    return True
                return False

            # ---------------- phase 1 chunk waves + slots ----------------
            for pi, c in enumerate(chunk_order):
                nxt_c = chunk_order[pi + 1] if pi + 1 < NCH else None
                # qk waves: one e-tile each (0,1 = q_h0,q_h1; 2,3 = k_h0,k_h1)
                dsts = [qt_sb[0], qt_sb[1], kt_sb[0], kt_sb[1]]
                if pi == 0:
                    # first chunk: interleaved eighth DMAs so the first
                    # matmuls aren't waiting on one 2MB transfer
                    t = xt_pool.tile([128, NDT * 512], bf16, tag="xt")
                    xt_tiles[c] = t
                    w = NDT * 512 // 4
                    for i in range(4):
                        nc.sync.dma_start(
                            wqk_sb[:, i * w:(i + 1) * w],
                            wqk_d.ap()[:, i * w:(i + 1) * w])
                        nc.sync.dma_start(
                            t[:, i * w:(i + 1) * w],
                            xt_d.ap()[:, c * NDT * 512 + i * w:
                                      c * NDT * 512 + (i + 1) * w])
                    nc.sync.dma_start(wv_sb[:], wv_d.ap()[:, :])
                    for h in range(HPC):
                        nc.sync.dma_start(
                            wo_sb[h][:],
                            wo_d.ap()[h * 128:(h + 1) * 128, :])
                xt_t = xt_tile(c)
                for e in range(4):
                    qk_ps = qk_ps_pool.tile([128, 512], f32, tag="qk",
                                            name="qkps")
                    for kd in range(NDT):
                        if e == 2 and nxt_c is not None and kd == 0:
                            xt_tile(nxt_c)          # prefetch next chunk
                        nc.tensor.matmul(
                            qk_ps[:],
                            wqk_sb[:, kd * 512 + e * 128:
                                   kd * 512 + (e + 1) * 128],
                            xt_t[:, kd * 512:(kd + 1) * 512],
                            start=kd == 0, stop=kd == NDT - 1)
                        if kd % 4 == 1 or kd == 15:
                            fill_slot()
                    nc.vector.tensor_copy(
                        dsts[e][:, c * 512:(c + 1) * 512], qk_ps[:])
                qk_done.add(c)
                # v waves: 2 token-tiles per wave, one PSUM bank each
                for wave in range(2):
                    v_ps = [v_ps_pool.tile([128, 256], f32, tag="v",
                                           name="vps")
                            for _ in range(2)]
                    for kd in range(NDT):
                        st, sp = kd == 0, kd == NDT - 1
                        for t2 in range(2):
                            tl = wave * 2 + t2
                            nc.tensor.matmul(
                                v_ps[t2][:],
                                xt_t[:, kd * 512 + tl * 128:
                                     kd * 512 + (tl + 1) * 128],
                                wv_sb[:, kd * 256:(kd + 1) * 256],
                                start=st, stop=sp)
                        if kd % 2 == 1:
                            fill_slot()
                    for t2 in range(2):
                        tok = c * 4 + wave * 2 + t2
                        nc.scalar.copy(
                            v_sb[:, tok * 256:(tok + 1) * 256], v_ps[t2][:])
                chunks_done.add(c)

            # ---------------- drain: attention + projection ----------------
            # phase-1 PSUM banks freed -> extra s/y pools for pipelining
            p1.close()
            s2_pool = stack.enter_context(
                tc.tile_pool(name="ps_s2", bufs=1, space="PSUM"))
            y2_pool = stack.enter_context(
                tc.tile_pool(name="ps_y2", bufs=1, space="PSUM"))
            s_pools.append(s2_pool)
            y_pools.append(y2_pool)
            while fill_slot():
                pass

            if dump:
                dsts = [qt_sb[0], qt_sb[1], kt_sb[0], kt_sb[1]]
                for e in range(4):
                    nc.sync.dma_start(
                        qk_dump.ap()[e * 128:(e + 1) * 128, :], dsts[e][:])
                nc.sync.dma_start(v_dump.ap()[:, :], v_sb[:])
                for h in range(HPC):
                    nc.sync.dma_start(
                        at_dump.ap()[h * 128:(h + 1) * 128, :], at_sb[h][:])

    nc.compile()
    return nc


last_results = None  # set when KERNEL_TRACE=1


def kernel(x, mask, W_qkv, W_o):
    import os
    import ml_dtypes
    from concourse.bass_utils import run_bass_kernel_spmd

    bf = ml_dtypes.bfloat16
    x = np.asarray(x, dtype=np.float32)
    mask_np = np.asarray(mask).astype(bool)
    W_qkv = np.asarray(W_qkv, dtype=np.float32)
    W_o = np.asarray(W_o, dtype=np.float32)
    m2 = np.broadcast_to(mask_np, (1, 1, S, S))[0, 0]

    key = m2.tobytes()
    nc = _cache.get(key)
    if nc is None:
        nc = _build(m2)
        _cache[key] = nc

    # xt packed: [p, (c*16+kd)*512 + t] = x^T[kd*128+p, c*512+t]
    xt_full = x.reshape(T, D).T                                  # [D, T]
    xtp = np.ascontiguousarray(
        xt_full.reshape(NDT, 128, NCH, 512).transpose(1, 2, 0, 3)
        .reshape(128, NCH * NDT * 512)).astype(bf)
    mt = np.ascontiguousarray(m2.T.astype(np.float32)).astype(bf)

    in_maps = []
    for c in range(NCORES):
        hA, hB = HPC * c, HPC * c + 1
        q_rows = list(range(hA * DK, (hA + 1) * DK)) + \
                 list(range(hB * DK, (hB + 1) * DK))
        k_rows = [D + r for r in q_rows]
        v_rows = [2 * D + r for r in q_rows]
        wqk = W_qkv[q_rows + k_rows, :].T                        # [D, 512]
        # packed: [p, kd*512 + col] = wqk[kd*128+p, col]
        wqkp = np.ascontiguousarray(
            wqk.reshape(NDT, 128, 512).transpose(1, 0, 2)
            .reshape(128, NDT * 512)).astype(bf)
        wv = W_qkv[v_rows, :].T                                  # [D, 256]
        wvp = np.ascontiguousarray(
            wv.reshape(NDT, 128, 256).transpose(1, 0, 2)
            .reshape(128, NDT * 256)).astype(bf)
        wo = np.ascontiguousarray(W_o[:, q_rows].T).astype(bf)
        in_maps.append({"xt": xtp, "wqk": wqkp, "wv": wvp, "wo": wo,
                        "mt": mt})

    trace = bool(os.environ.get("KERNEL_TRACE"))
    res = run_bass_kernel_spmd(nc, in_maps, core_ids=list(range(NCORES)),
                               trace=trace)
    if trace:
        global last_results
        last_results = res
    y = res.results[0]["y"].astype(np.float32)
    for c in range(1, NCORES):
        y += res.results[c]["y"].astype(np.float32)
    return y.reshape(B, S, D)


# revision 19
# speedup vs baseline: 1.0057x; 1.0057x over previous
"""Multi-head causal attention on 8 Trainium2 NeuronCores - v2.

Tensor-parallel over heads (2/core). Single interleaved PE instruction
stream: QKV projection chunk waves, attention units and out-projection
units are emitted into one dense sequence so the PE never idles, HAM
stays warm, and ACT/DVE softmax work hides under matmuls.

Data is bf16 (inputs, Q/K/V, exp(S), attention out, W_o, y partials);
PSUM accumulation stays f32. Softmax denominator is computed on the PE:
db[128,512] += ones128.T @ e per entry (clean pairs pre-summed on DVE
at 2x bf16 rate), so no partition-reduce chains and no [1,512] ops;
1/d via reciprocal_approx_fast on [128,512], one DVE multiply
(o_ps PSUM operand) produces the normalized attention out.

PSUM budget (8 banks): qk 2 + v 1 + s 2 + o 1 + db 1 + y 1.
"""
import sys
if '/opt/trn_rl_repo' not in sys.path:
    sys.path.insert(0, '/opt/trn_rl_repo')

import numpy as np

B, S, D = 2, 2048, 2048
H, DK = 16, 128
NCORES = 8
HPC = H // NCORES            # heads per core
T = B * S                    # tokens
QB = 512                     # q-block width
NKT = S // 128               # k tiles per batch (16)
NQB = S // QB                # q blocks per batch (4)
NCH = T // QB                # token chunks (8)
NDT = D // 128               # d_model tiles (16)

_cache = {}


def _analyze_mask(m2):
    """m2: [S, S] bool. Returns blocks[qb] = list of entries
    (j, q0, m0, m1) ascending j: q0 first valid col (block-local),
    m0..m1 mask-multiply range (None if fully valid from q0)."""
    blocks = []
    for qb in range(NQB):
        entries = []
        for j in range(NKT):
            blk = m2[qb * QB:(qb + 1) * QB, j * 128:(j + 1) * 128]
            col_any = blk.any(axis=1)
            if not col_any.any():
                continue
            col_all = blk.all(axis=1)
            q0 = int(np.argmax(col_any))
            rev = col_all[::-1]
            run = int(np.argmin(rev)) if not rev.all() else QB
            q1 = QB - run
            if q1 <= q0:
                entries.append((j, q0, None, None))
            else:
                entries.append((j, q0, q0, q1))
        blocks.append(entries)
    return blocks


def _build(mask_bool):
    from contextlib import ExitStack
    import concourse.bass as bass
    import concourse.tile as tile
    from concourse import bacc, mybir

    f32 = mybir.dt.float32
    f32r = mybir.dt.float32r
    bf16 = mybir.dt.bfloat16
    EXP = mybir.ActivationFunctionType.Exp
    scale = 1.0 / np.sqrt(DK)

    m2 = mask_bool
    blocks = _analyze_mask(m2)
    for ents in blocks:
        assert ents and ents[0][1] == min(e[1] for e in ents), \
            "first entry must cover the widest q range"

    nc = bacc.Bacc("TRN2", target_bir_lowering=False, debug=False)
    # xt packed host-side as [128, (c*16+kd)*512 + t] so one DMA per chunk
    xt_d = nc.dram_tensor("xt", [128, NCH * NDT * 512], bf16,
                          kind="ExternalInput")
    # wqk packed as [128, kd*512 + e*128 + col]; wv as [128, kd*256 + col]
    wqk_d = nc.dram_tensor("wqk", [128, NDT * 512], bf16,
                           kind="ExternalInput")
    wv_d = nc.dram_tensor("wv", [128, NDT * 256], bf16,
                          kind="ExternalInput")
    wo_d = nc.dram_tensor("wo", [2 * 128, D], bf16, kind="ExternalInput")
    mt_d = nc.dram_tensor("mt", [S, S], bf16, kind="ExternalInput")
    y_d = nc.dram_tensor("y", [T, D], bf16, kind="ExternalOutput")
    import os as _os
    dump = bool(_os.environ.get("KERNEL_DUMP"))
    if dump:
        qk_dump = nc.dram_tensor("qk_dump", [512, T], bf16,
                                 kind="ExternalOutput")
        v_dump = nc.dram_tensor("v_dump", [128, (T // 128) * 256], bf16,
                                kind="ExternalOutput")
        at_dump = nc.dram_tensor("at_dump", [256, T], bf16,
                                 kind="ExternalOutput")
        d_dump = nc.dram_tensor("d_dump", [128, 512], f32,
                                kind="ExternalOutput")

    with tile.TileContext(nc) as tc:
        with ExitStack() as stack:
            stack.enter_context(
                nc.allow_low_precision(reason="bf16 kernel"))
            qkt_pool = stack.enter_context(tc.tile_pool(name="qkt", bufs=1))
            v_pool = stack.enter_context(tc.tile_pool(name="vsb", bufs=1))
            att_pool = stack.enter_context(tc.tile_pool(name="att", bufs=1))
            cst_pool = stack.enter_context(tc.tile_pool(name="cst", bufs=1))
            w_pool = stack.enter_context(tc.tile_pool(name="wts", bufs=1))
            xt_pool = stack.enter_context(tc.tile_pool(name="xt", bufs=3))
            e_pool = stack.enter_context(tc.tile_pool(name="e", bufs=8))
            es_pool = stack.enter_context(tc.tile_pool(name="es", bufs=6))
            acc_pool = stack.enter_context(tc.tile_pool(name="acc", bufs=3))
            rcp_pool = stack.enter_context(tc.tile_pool(name="rcp", bufs=2))
            msk_pool = stack.enter_context(tc.tile_pool(name="msk", bufs=1))
            ysb_pool = stack.enter_context(tc.tile_pool(name="ysb", bufs=4))

            s_ps_pool = stack.enter_context(
                tc.tile_pool(name="ps_s", bufs=1, space="PSUM"))
            o_ps_pool = stack.enter_context(
                tc.tile_pool(name="ps_o", bufs=1, space="PSUM"))
            db_ps_pool = stack.enter_context(
                tc.tile_pool(name="ps_db", bufs=1, space="PSUM"))
            y_ps_pool = stack.enter_context(
                tc.tile_pool(name="ps_y", bufs=1, space="PSUM"))
            # phase-1 pools opened last so they can close before the drain
            p1 = ExitStack()
            qk_ps_pool = p1.enter_context(
                tc.tile_pool(name="ps_qk", bufs=1, space="PSUM"))
            v_ps_pool = p1.enter_context(
                tc.tile_pool(name="ps_v", bufs=2, space="PSUM"))
            # drain-phase extra pools (opened after phase-1 pools close)
            s_pools = [s_ps_pool]
            y_pools = [y_ps_pool]

            # ------------ persistent SBUF ------------
            qt_sb = [qkt_pool.tile([128, T], bf16, tag=f"qt{h}", name=f"qt{h}")
                     for h in range(HPC)]
            kt_sb = [qkt_pool.tile([128, T], bf16, tag=f"kt{h}", name=f"kt{h}")
                     for h in range(HPC)]
            v_sb = v_pool.tile([128, (T // 128) * 256], bf16, tag="v")
            at_sb = [att_pool.tile([128, T], bf16, tag=f"at{h}", name=f"at{h}")
                     for h in range(HPC)]

            ones_f = cst_pool.tile([128, 128], f32, tag="ones_f")
            nc.vector.memset(ones_f[:], 1.0)
            ones_fr = cst_pool.tile([128, 128], f32r, tag="ones_fr")
            nc.scalar.copy(ones_fr[:], ones_f[:])
            # warm up the exp table early (ACT_TABLE_LOAD ~2.7us)
            exp_warm = cst_pool.tile([1, 1], f32, tag="expw")
            nc.scalar.activation(exp_warm[:], ones_f[0:1, 0:1], EXP)

            # ------------ weights: few large DMAs ------
            wqk_sb = w_pool.tile([128, NDT * 512], bf16, tag="wqk")
            wv_sb = w_pool.tile([128, NDT * 256], bf16, tag="wv")
            wo_sb = []
            for h in range(HPC):
                wt = w_pool.tile([128, D], bf16, tag=f"wo{h}", name=f"wo{h}")
                wo_sb.append(wt)

            # xt tiles: one [128, 16*512] per chunk (cols kd*512 + t)
            xt_tiles = {}

            def xt_tile(c, split=1):
                t = xt_tiles.get(c)
                if t is None:
                    t = xt_pool.tile([128, NDT * 512], bf16, tag="xt")
                    w = NDT * 512 // split
                    for i in range(split):
                        # gpsimd queue: it is otherwise idle, so xt DMA
                        # issue runs parallel to sync-queue traffic
                        nc.sync.dma_start(
                            t[:, i * w:(i + 1) * w],
                            xt_d.ap()[:, c * NDT * 512 + i * w:
                                      c * NDT * 512 + (i + 1) * w])
                    xt_tiles[c] = t
                return t

            # mask tile cache keyed by content
            mask_tiles = {}

            def mask_tile(j, qb, m0, m1):
                key = m2[qb * QB + m0:qb * QB + m1,
                         j * 128:(j + 1) * 128].tobytes()
                t = mask_tiles.get(key)
                if t is None:
                    t = msk_pool.tile([128, QB], bf16,
                                      name=f"mask{len(mask_tiles)}",
                                      tag=f"m{len(mask_tiles)}")
                    nc.sync.dma_start(
                        t[:, 0:m1 - m0],
                        mt_d.ap()[j * 128:(j + 1) * 128,
                                  qb * QB + m0:qb * QB + m1])
                    mask_tiles[key] = t
                return t

            # ---------------- attention stream ----------------
            class Stream:
                """One (b, h, qb): units alternate S/exp and d/PV."""

                def __init__(self, b, h, qb):
                    self.b, self.h, self.qb = b, h, qb
                    self.tb = b * S
                    ents = blocks[qb]
                    self.groups = [(ents[i:i + 2], i)
                                   for i in range(0, len(ents), 2)]
                    assert ents[0][1] == 0, "first entry must cover q0=0"
                    self.ne = len(ents)
                    self.qcol = self.tb + qb * QB
                    self.o_ps = None
                    self.es_acc = None
                    self.gi = 0
                    self.pends = []
                    self.done_units = False
                    self.inline_proj = False

                def unit_s(self, grp):
                    """S matmuls + exp + mask + denominator chain."""
                    h, tb = self.h, self.tb
                    q0g = grp[0][1]
                    n = len(grp)
                    sp = s_pools[sched['sc'] % len(s_pools)]
                    sched['sc'] += 1
                    s_t = sp.tile([128, 2 * QB], f32, tag="s",
                                  name="sps")
                    for idx, (j, q0, m0, m1) in enumerate(grp):
                        nc.tensor.matmul(
                            s_t[:, idx * QB + q0:(idx + 1) * QB],
                            kt_sb[h][:, tb + j * 128:tb + (j + 1) * 128],
                            qt_sb[h][:, self.qcol + q0:self.qcol + QB],
                            start=True, stop=True)
                    e_t = e_pool.tile([128, 2 * QB], bf16, tag="e",
                                      name="esb")
                    # one exp over [q0g, n*QB): start=True zeroed the
                    # whole bank, so unwritten score cols read exp(0)=1
                    # and are zeroed by the extended mask below
                    nc.scalar.activation(e_t[:, q0g:n * QB],
                                         s_t[:, q0g:n * QB], EXP,
                                         scale=scale)
                    for idx, (j, q0, m0, m1) in enumerate(grp):
                        hi = m1 if m1 is not None else \
                            (q0 if q0 > q0g else None)
                        if hi is not None and hi > q0g:
                            mt = mask_tile(j, self.qb, q0g, hi)
                            nc.vector.tensor_mul(
                                e_t[:, idx * QB + q0g:idx * QB + hi],
                                e_t[:, idx * QB + q0g:idx * QB + hi],
                                mt[:, 0:hi - q0g])
                    # denominator chain: es = e0 + e1, acc += es (f32)
                    if n == 2:
                        es_t = es_pool.tile([128, QB], bf16, tag="es",
                                            name="essb")
                        nc.vector.tensor_add(es_t[:, q0g:QB],
                                             e_t[:, q0g:QB],
                                             e_t[:, QB + q0g:2 * QB])
                        es_ap = es_t[:, q0g:QB]
                    else:
                        es_ap = e_t[:, q0g:QB]
                    if self.es_acc is None:
                        self.es_acc = acc_pool.tile([128, QB], f32r,
                                                    tag="acc", name="acc")
                        nc.vector.tensor_copy(self.es_acc[:, q0g:QB],
                                              es_ap)
                    else:
                        nc.vector.tensor_add(self.es_acc[:, q0g:QB],
                                             self.es_acc[:, q0g:QB],
                                             es_ap)
                    return (grp, e_t)

                def unit_pv(self, pend, g0):
                    """PV matmuls for a completed group."""
                    grp, e_t = pend
                    b, h = self.b, self.h
                    if self.o_ps is None:
                        self.o_ps = o_ps_pool.tile([128, QB], f32,
                                                   tag="o", name="ops")
                    for idx, (j, q0, m0, m1) in enumerate(grp):
                        gi = g0 + idx
                        nc.tensor.matmul(
                            self.o_ps[:, q0:QB],
                            v_sb[:, (b * NKT + j) * 256 + h * 128:
                                 (b * NKT + j) * 256 + (h + 1) * 128],
                            e_t[:, idx * QB + q0:(idx + 1) * QB],
                            start=(gi == 0), stop=(gi == self.ne - 1))

                def unit_tail(self):
                    if (self.h == HPC - 1
                            and (self.b, self.qb) == last_bqb):
                        # final stream: pipeline the tail per token-tile
                        # with inline projection (nothing else is left to
                        # hide the serial d->recip->mul chain behind)
                        db_ps = db_ps_pool.tile([128, QB], f32,
                                                tag="db", name="dbps")
                        rcp = rcp_pool.tile([128, QB], f32, tag="rcp",
                                            name="rcp")
                        for q4 in range(4):
                            sl = slice(q4 * 128, (q4 + 1) * 128)
                            nc.tensor.matmul(
                                db_ps[:, sl], ones_fr[:],
                                self.es_acc[:, sl],
                                start=q4 == 0, stop=q4 == 3)
                            nc.vector.reciprocal_approx_fast(
                                rcp[:, sl], db_ps[:, sl])
                            nc.vector.tensor_mul(
                                at_sb[self.h][:, self.qcol + q4 * 128:
                                              self.qcol + (q4 + 1) * 128],
                                self.o_ps[:, sl], rcp[:, sl])
                            for ch in range(4):
                                do_proj(self.b, self.qb * 4 + q4, ch)
                        self.inline_proj = True
                        return
                    db_ps = db_ps_pool.tile([128, QB], f32,
                                            tag="db", name="dbps")
                    nc.tensor.matmul(db_ps[:], ones_fr[:],
                                     self.es_acc[:], start=True,
                                     stop=True)
                    if dump and self.b == 0 and self.h == 0 and self.qb == 0:
                        dtmp = rcp_pool.tile([128, QB], f32, tag="dtmp",
                                             name="dtmp")
                        nc.vector.tensor_copy(dtmp[:], db_ps[:])
                        nc.sync.dma_start(d_dump.ap()[:, :], dtmp[:])
                    rcp = rcp_pool.tile([128, QB], f32, tag="rcp",
                                        name="rcp")
                    nc.vector.reciprocal_approx_fast(rcp[:], db_ps[:])
                    nc.vector.tensor_mul(
                        at_sb[self.h][:, self.qcol:self.qcol + QB],
                        self.o_ps[:], rcp[:])

                def step(self, s_only=False):
                    """Emit one unit. s_only: S/exp lookahead without PV
                    (safe while an older stream still owns the o bank)."""
                    if s_only:
                        if (self.gi < len(self.groups)
                                and len(self.pends) < 2):
                            grp, g0 = self.groups[self.gi]
                            self.pends.append((self.unit_s(grp), g0))
                            self.gi += 1
                            return True
                        return False
                    if self.gi < len(self.groups):
                        grp, g0 = self.groups[self.gi]
                        nxt = self.unit_s(grp)
                        if self.pends:
                            pend, pg = self.pends.pop(0)
                            self.unit_pv(pend, pg)
                        self.pends.append((nxt, g0))
                        self.gi += 1
                        return True
                    if self.pends:
                        pend, pg = self.pends.pop(0)
                        self.unit_pv(pend, pg)
                        return True
                    if not self.done_units:
                        self.unit_tail()
                        self.done_units = True
                        return True
                    return False

            # ---------------- projection units ----------------
            proj_queue = []   # (b, tt, ch)

            # chunk emission order interleaves batches so attention work
            # is available from the second chunk onward
            chunk_order = [0, 4, 1, 5, 2, 6, 3, 7]
            ysb_tiles = {}
            last_bqb = (chunk_order[-1] // 4, chunk_order[-1] % 4)

            def emit_proj_unit():
                do_proj(*proj_queue.pop(0))

            def do_proj(b, tt, ch):
                trow = b * S + tt * 128
                yp = y_pools[sched['yc'] % len(y_pools)]
                sched['yc'] += 1
                y_ps = yp.tile([128, 512], f32, tag="y", name="yps")
                for hh in range(HPC):
                    nc.tensor.matmul(
                        y_ps[:],
                        at_sb[hh][:, trow:trow + 128],
                        wo_sb[hh][:, ch * 512:(ch + 1) * 512],
                        start=(hh == 0), stop=(hh == HPC - 1))
                y_sb = ysb_tiles.get((b, tt))
                if y_sb is None:
                    y_sb = ysb_pool.tile([128, D], bf16, tag="ysb",
                                         name="ysb")
                    ysb_tiles[(b, tt)] = y_sb
                if (tt + ch) % 2 == 0:
                    nc.scalar.copy(y_sb[:, ch * 512:(ch + 1) * 512],
                                   y_ps[:])
                else:
                    nc.vector.tensor_copy(y_sb[:, ch * 512:(ch + 1) * 512],
                                          y_ps[:])
                last_qb = (b, tt // 4) == (last_bqb[0], last_bqb[1])
                if last_qb:
                    # final q-block: per-column-block DMA so the output
                    # transfers start before the kernel tail
                    nc.sync.dma_start(
                        y_d.ap()[trow:trow + 128,
                                 ch * 512:(ch + 1) * 512],
                        y_sb[:, ch * 512:(ch + 1) * 512])
                    if ch == 3:
                        del ysb_tiles[(b, tt)]
                elif ch == 3:
                    # all four column blocks done: one DMA per token row
                    nc.sync.dma_start(
                        y_d.ap()[trow:trow + 128, :], y_sb[:])
                    del ysb_tiles[(b, tt)]

            # ---------------- scheduler ----------------
            stream_list = []    # in ready order with chunk gates
            for qb in range(NQB):
                for b in range(B):
                    for h in range(HPC):
                        stream_list.append((4 * b + qb, Stream(b, h, qb)))
            sched = {'si': 0, 'tog': False, 'sc': 0, 'yc': 0, 'rr': 0}
            chunks_done = set()
            qk_done = set()     # chunk's q/k ready (before its v waves)

            def cur_stream():
                si = sched['si']
                if si >= len(stream_list):
                    return None
                gate, st = stream_list[si]
                if gate not in chunks_done:
                    return None
                return st

            def stream_done(st):
                return (st.done_units and not st.pends
                        and st.gi >= len(st.groups))

            def step_stream():
                # lead stream runs normally; the next stream may prefetch
                # S/exp units (no PV) so ACT stays fed during tails
                base = sched['si']
                rr = sched['rr']
                sched['rr'] = 1 - rr
                cands = [(base, False), (base + 1, True)]
                if rr:
                    cands.reverse()
                for si, s_only in cands:
                    if si >= len(stream_list):
                        continue
                    gate, st = stream_list[si]
                    if gate not in qk_done or stream_done(st):
                        continue
                    if gate not in chunks_done:
                        s_only = True   # v not ready: S/exp units only
                    if not st.step(s_only=s_only):
                        continue
                    while sched['si'] < len(stream_list):
                        g2, s2 = stream_list[sched['si']]
                        if not stream_done(s2):
                            break
                        if s2.h == HPC - 1 and not s2.inline_proj:
                            for t4 in range(4):
                                for ch in range(4):
                                    proj_queue.append(
                                        (s2.b, s2.qb * 4 + t4, ch))
                        sched['si'] += 1
                    return True
                return False

            def fill_slot():
                # alternate stream units and projection units; drain the
                # proj queue faster when it backs up
                tog = sched['tog']
                sched['tog'] = not tog
                if tog and proj_queue:
                    emit_proj_unit()
                    return True
                if step_stream():
                    return True
                if proj_queue:
                    emit_proj_unit()
                    return True
                return False

            # ---------------- phase 1 chunk waves + slots ----------------
            for pi, c in enumerate(chunk_order):
                nxt_c = chunk_order[pi + 1] if pi + 1 < NCH else None
                # qk waves: one e-tile each (0,1 = q_h0,q_h1; 2,3 = k_h0,k_h1)
                dsts = [qt_sb[0], qt_sb[1], kt_sb[0], kt_sb[1]]
                if pi == 0:
                    # first chunk: interleaved eighth DMAs so the first
                    # matmuls aren't waiting on one 2MB transfer
                    t = xt_pool.tile([128, NDT * 512], bf16, tag="xt")
                    xt_tiles[c] = t
                    w = NDT * 512 // 8
                    for i in range(8):
                        nc.sync.dma_start(
                            wqk_sb[:, i * w:(i + 1) * w],
                            wqk_d.ap()[:, i * w:(i + 1) * w])
                        nc.sync.dma_start(
                            t[:, i * w:(i + 1) * w],
                            xt_d.ap()[:, c * NDT * 512 + i * w:
                                      c * NDT * 512 + (i + 1) * w])
                    nc.sync.dma_start(wv_sb[:], wv_d.ap()[:, :])
                    for h in range(HPC):
                        nc.sync.dma_start(
                            wo_sb[h][:],
                            wo_d.ap()[h * 128:(h + 1) * 128, :])
                xt_t = xt_tile(c)
                for e in range(4):
                    qk_ps = qk_ps_pool.tile([128, 512], f32, tag="qk",
                                            name="qkps")
                    for kd in range(NDT):
                        if e == 2 and nxt_c is not None and kd == 0:
                            xt_tile(nxt_c)          # prefetch next chunk
                        nc.tensor.matmul(
                            qk_ps[:],
                            wqk_sb[:, kd * 512 + e * 128:
                                   kd * 512 + (e + 1) * 128],
                            xt_t[:, kd * 512:(kd + 1) * 512],
                            start=kd == 0, stop=kd == NDT - 1)
                        if kd % 4 == 1:
                            fill_slot()
                    nc.vector.tensor_copy(
                        dsts[e][:, c * 512:(c + 1) * 512], qk_ps[:])
                qk_done.add(c)
                # v waves: 2 token-tiles per wave, one PSUM bank each
                for wave in range(2):
                    v_ps = [v_ps_pool.tile([128, 256], f32, tag="v",
                                           name="vps")
                            for _ in range(2)]
                    for kd in range(NDT):
                        st, sp = kd == 0, kd == NDT - 1
                        for t2 in range(2):
                            tl = wave * 2 + t2
                            nc.tensor.matmul(
                                v_ps[t2][:],
                                xt_t[:, kd * 512 + tl * 128:
                                     kd * 512 + (tl + 1) * 128],
                                wv_sb[:, kd * 256:(kd + 1) * 256],
                                start=st, stop=sp)
                        if kd % 2 == 1:
                            fill_slot()
                    for t2 in range(2):
                        tok = c * 4 + wave * 2 + t2
                        nc.scalar.copy(
                            v_sb[:, tok * 256:(tok + 1) * 256], v_ps[t2][:])
                chunks_done.add(c)

            # ---------------- drain: attention + projection ----------------
            # phase-1 PSUM banks freed -> extra s/y pools for pipelining
            p1.close()
            s2_pool = stack.enter_context(
                tc.tile_pool(name="ps_s2", bufs=1, space="PSUM"))
            y2_pool = stack.enter_context(
                tc.tile_pool(name="ps_y2", bufs=1, space="PSUM"))
            s_pools.append(s2_pool)
            y_pools.append(y2_pool)
            while fill_slot():
                pass

            if dump:
                dsts = [qt_sb[0], qt_sb[1], kt_sb[0], kt_sb[1]]
                for e in range(4):
                    nc.sync.dma_start(
                        qk_dump.ap()[e * 128:(e + 1) * 128, :], dsts[e][:])
                nc.sync.dma_start(v_dump.ap()[:, :], v_sb[:])
                for h in range(HPC):
                    nc.sync.dma_start(
                        at_dump.ap()[h * 128:(h + 1) * 128, :], at_sb[h][:])

    nc.compile()
    return nc


last_results = None  # set when KERNEL_TRACE=1


def kernel(x, mask, W_qkv, W_o):
    import os
    import ml_dtypes
    from concourse.bass_utils import run_bass_kernel_spmd

    bf = ml_dtypes.bfloat16
    x = np.asarray(x, dtype=np.float32)
    mask_np = np.asarray(mask).astype(bool)
    W_qkv = np.asarray(W_qkv, dtype=np.float32)
    W_o = np.asarray(W_o, dtype=np.float32)
    m2 = np.broadcast_to(mask_np, (1, 1, S, S))[0, 0]

    key = m2.tobytes()
    nc = _cache.get(key)
    if nc is None:
        nc = _build(m2)
        _cache[key] = nc

    # xt packed: [p, (c*16+kd)*512 + t] = x^T[kd*128+p, c*512+t]
    xt_full = x.reshape(T, D).T                                  # [D, T]
    xtp = np.ascontiguousarray(
        xt_full.reshape(NDT, 128, NCH, 512).transpose(1, 2, 0, 3)
        .reshape(128, NCH * NDT * 512)).astype(bf)
    mt = np.ascontiguousarray(m2.T.astype(np.float32)).astype(bf)

    in_maps = []
    for c in range(NCORES):
        hA, hB = HPC * c, HPC * c + 1
        q_rows = list(range(hA * DK, (hA + 1) * DK)) + \
                 list(range(hB * DK, (hB + 1) * DK))
        k_rows = [D + r for r in q_rows]
        v_rows = [2 * D + r for r in q_rows]
        wqk = W_qkv[q_rows + k_rows, :].T                        # [D, 512]
        # packed: [p, kd*512 + col] = wqk[kd*128+p, col]
        wqkp = np.ascontiguousarray(
            wqk.reshape(NDT, 128, 512).transpose(1, 0, 2)
            .reshape(128, NDT * 512)).astype(bf)
        wv = W_qkv[v_rows, :].T                                  # [D, 256]
        wvp = np.ascontiguousarray(
            wv.reshape(NDT, 128, 256).transpose(1, 0, 2)
            .reshape(128, NDT * 256)).astype(bf)
        wo = np.ascontiguousarray(W_o[:, q_rows].T).astype(bf)
        in_maps.append({"xt": xtp, "wqk": wqkp, "wv": wvp, "wo": wo,
                        "mt": mt})

    trace = bool(os.environ.get("KERNEL_TRACE"))
    res = run_bass_kernel_spmd(nc, in_maps, core_ids=list(range(NCORES)),
                               trace=trace)
    if trace:
        global last_results
        last_results = res
    y = res.results[0]["y"].astype(np.float32)
    for c in range(1, NCORES):
        y += res.results[c]["y"].astype(np.float32)
    return y.reshape(B, S, D)


# revision 22
# speedup vs baseline: 1.0243x; 1.0185x over previous
"""Multi-head causal attention on 8 Trainium2 NeuronCores.

Tensor-parallel over heads (2/core), x replicated, y = host-sum of
per-core partials. One interleaved PE instruction stream: QKV
projection chunk waves (batch-interleaved chunk order 0,4,1,5,...),
attention units and out-projection units share the queue so the PE
never idles >~1us, HAM stays at the warm 2.4 GHz clock, and exp/DVE
softmax work hides under matmuls.

All matmul data is bf16 (x, W_qkv, W_o, Q/K/V, exp(S), attn out, y
partials; PSUM accumulation f32), which halves DMA/SBUF and enables
fast weight loads; rel err ~4e-3 vs the f32 reference (budget 2e-2).
Host packs x^T/weights into SBUF-layout rows so phase-1 needs one DMA
per chunk (Sync DIRECT2D issue rate, ~0.6us per descriptor, was the
startup bottleneck).

Softmax: scores are computed transposed (S^T = K_tile^T . Q block) so
exp(S^T) feeds P.V directly; no max-subtraction (scores are O(1) by
construction). Denominator: per 2-entry group one DVE pair-add, then
an f32 running-sum chain per stream; one ones[128x128] matmul at the
stream tail broadcasts d to [128,512], reciprocal_approx_fast (custom
DVE op, ~0.7us) and one PSUM-operand multiply produce the normalized
attention out. The final stream splits its tail per token-tile with
inline projection to pipeline the end of the kernel.

PSUM budget (8 banks): qk 1 + v 2 + s 2 + o 1 + db 1 + y 1; after the
chunk waves the qk/v pools close and a second s pool + y pool open for
deeper drain pipelining.

Measured: ~325-330 us per core (8 cores SPMD), vs 472 us baseline.
"""
import sys
if '/opt/trn_rl_repo' not in sys.path:
    sys.path.insert(0, '/opt/trn_rl_repo')

import numpy as np

B, S, D = 2, 2048, 2048
H, DK = 16, 128
NCORES = 8
HPC = H // NCORES            # heads per core
T = B * S                    # tokens
QB = 512                     # q-block width
NKT = S // 128               # k tiles per batch (16)
NQB = S // QB                # q blocks per batch (4)
NCH = T // QB                # token chunks (8)
NDT = D // 128               # d_model tiles (16)

_cache = {}


def _analyze_mask(m2):
    """m2: [S, S] bool. Returns blocks[qb] = list of entries
    (j, q0, m0, m1) ascending j: q0 first valid col (block-local),
    m0..m1 mask-multiply range (None if fully valid from q0)."""
    blocks = []
    for qb in range(NQB):
        entries = []
        for j in range(NKT):
            blk = m2[qb * QB:(qb + 1) * QB, j * 128:(j + 1) * 128]
            col_any = blk.any(axis=1)
            if not col_any.any():
                continue
            col_all = blk.all(axis=1)
            q0 = int(np.argmax(col_any))
            rev = col_all[::-1]
            run = int(np.argmin(rev)) if not rev.all() else QB
            q1 = QB - run
            if q1 <= q0:
                entries.append((j, q0, None, None))
            else:
                entries.append((j, q0, q0, q1))
        blocks.append(entries)
    return blocks


def _build(mask_bool):
    from contextlib import ExitStack
    import concourse.bass as bass
    import concourse.tile as tile
    from concourse import bacc, mybir

    f32 = mybir.dt.float32
    f32r = mybir.dt.float32r
    bf16 = mybir.dt.bfloat16
    EXP = mybir.ActivationFunctionType.Exp
    scale = 1.0 / np.sqrt(DK)

    m2 = mask_bool
    blocks = _analyze_mask(m2)
    for ents in blocks:
        assert ents and ents[0][1] == min(e[1] for e in ents), \
            "first entry must cover the widest q range"

    nc = bacc.Bacc("TRN2", target_bir_lowering=False, debug=False)
    # xt packed host-side as [128, (c*16+kd)*512 + t] so one DMA per chunk
    xt_d = nc.dram_tensor("xt", [128, NCH * NDT * 512], bf16,
                          kind="ExternalInput")
    # wqk packed as [128, kd*512 + e*128 + col]; wv as [128, kd*256 + col]
    wqk_d = nc.dram_tensor("wqk", [128, NDT * 512], bf16,
                           kind="ExternalInput")
    wv_d = nc.dram_tensor("wv", [128, NDT * 256], bf16,
                          kind="ExternalInput")
    wo_d = nc.dram_tensor("wo", [2 * 128, D], bf16, kind="ExternalInput")
    mt_d = nc.dram_tensor("mt", [S, S], bf16, kind="ExternalInput")
    y_d = nc.dram_tensor("y", [T, D], bf16, kind="ExternalOutput")
    import os as _os
    dump = bool(_os.environ.get("KERNEL_DUMP"))
    if dump:
        qk_dump = nc.dram_tensor("qk_dump", [512, T], bf16,
                                 kind="ExternalOutput")
        v_dump = nc.dram_tensor("v_dump", [128, (T // 128) * 256], bf16,
                                kind="ExternalOutput")
        at_dump = nc.dram_tensor("at_dump", [256, T], bf16,
                                 kind="ExternalOutput")
        d_dump = nc.dram_tensor("d_dump", [128, 512], f32,
                                kind="ExternalOutput")

    with tile.TileContext(nc) as tc:
        with ExitStack() as stack:
            stack.enter_context(
                nc.allow_low_precision(reason="bf16 kernel"))
            qkt_pool = stack.enter_context(tc.tile_pool(name="qkt", bufs=1))
            v_pool = stack.enter_context(tc.tile_pool(name="vsb", bufs=1))
            att_pool = stack.enter_context(tc.tile_pool(name="att", bufs=1))
            cst_pool = stack.enter_context(tc.tile_pool(name="cst", bufs=1))
            w_pool = stack.enter_context(tc.tile_pool(name="wts", bufs=1))
            xt_pool = stack.enter_context(tc.tile_pool(name="xt", bufs=3))
            e_pool = stack.enter_context(tc.tile_pool(name="e", bufs=10))
            es_pool = stack.enter_context(tc.tile_pool(name="es", bufs=6))
            acc_pool = stack.enter_context(tc.tile_pool(name="acc", bufs=4))
            rcp_pool = stack.enter_context(tc.tile_pool(name="rcp", bufs=2))
            msk_pool = stack.enter_context(tc.tile_pool(name="msk", bufs=1))
            ysb_pool = stack.enter_context(tc.tile_pool(name="ysb", bufs=4))

            s_ps_pool = stack.enter_context(
                tc.tile_pool(name="ps_s", bufs=1, space="PSUM"))
            o_ps_pool = stack.enter_context(
                tc.tile_pool(name="ps_o", bufs=1, space="PSUM"))
            db_ps_pool = stack.enter_context(
                tc.tile_pool(name="ps_db", bufs=1, space="PSUM"))
            y_ps_pool = stack.enter_context(
                tc.tile_pool(name="ps_y", bufs=1, space="PSUM"))
            # phase-1 pools opened last so they can close before the drain
            p1 = ExitStack()
            qk_ps_pool = p1.enter_context(
                tc.tile_pool(name="ps_qk", bufs=1, space="PSUM"))
            v_ps_pool = p1.enter_context(
                tc.tile_pool(name="ps_v", bufs=2, space="PSUM"))
            # drain-phase extra pools (opened after phase-1 pools close)
            s_pools = [s_ps_pool]
            y_pools = [y_ps_pool]

            # ------------ persistent SBUF ------------
            qt_sb = [qkt_pool.tile([128, T], bf16, tag=f"qt{h}", name=f"qt{h}")
                     for h in range(HPC)]
            kt_sb = [qkt_pool.tile([128, T], bf16, tag=f"kt{h}", name=f"kt{h}")
                     for h in range(HPC)]
            v_sb = v_pool.tile([128, (T // 128) * 256], bf16, tag="v")
            at_sb = [att_pool.tile([128, T], bf16, tag=f"at{h}", name=f"at{h}")
                     for h in range(HPC)]

            ones_f = cst_pool.tile([128, 128], f32, tag="ones_f")
            nc.vector.memset(ones_f[:], 1.0)
            ones_fr = cst_pool.tile([128, 128], f32r, tag="ones_fr")
            nc.scalar.copy(ones_fr[:], ones_f[:])
            # warm up the exp table early (ACT_TABLE_LOAD ~2.7us)
            exp_warm = cst_pool.tile([1, 1], f32, tag="expw")
            nc.scalar.activation(exp_warm[:], ones_f[0:1, 0:1], EXP)
            # HAM pre-warm: ~4us of dummy matmuls while the first DMAs
            # are in flight, so real matmuls start at the 2.4 GHz clock
            warm_ps = y_ps_pool.tile([128, 512], f32, tag="y",
                                      name="yps")
            for i in range(44):
                nc.tensor.matmul(warm_ps[:, 0:128], ones_fr[:],
                                 ones_fr[:], start=i == 0, stop=i == 43)

            # ------------ weights: few large DMAs ------
            wqk_sb = w_pool.tile([128, NDT * 512], bf16, tag="wqk")
            wv_sb = w_pool.tile([128, NDT * 256], bf16, tag="wv")
            wo_sb = []
            for h in range(HPC):
                wt = w_pool.tile([128, D], bf16, tag=f"wo{h}", name=f"wo{h}")
                wo_sb.append(wt)

            # xt tiles: one [128, 16*512] per chunk (cols kd*512 + t)
            xt_tiles = {}

            def xt_tile(c, split=1):
                t = xt_tiles.get(c)
                if t is None:
                    t = xt_pool.tile([128, NDT * 512], bf16, tag="xt")
                    w = NDT * 512 // split
                    for i in range(split):
                        # gpsimd queue: it is otherwise idle, so xt DMA
                        # issue runs parallel to sync-queue traffic
                        nc.sync.dma_start(
                            t[:, i * w:(i + 1) * w],
                            xt_d.ap()[:, c * NDT * 512 + i * w:
                                      c * NDT * 512 + (i + 1) * w])
                    xt_tiles[c] = t
                return t

            # mask tile cache keyed by content
            mask_tiles = {}

            def mask_tile(j, qb, m0, m1):
                key = m2[qb * QB + m0:qb * QB + m1,
                         j * 128:(j + 1) * 128].tobytes()
                t = mask_tiles.get(key)
                if t is None:
                    t = msk_pool.tile([128, QB], bf16,
                                      name=f"mask{len(mask_tiles)}",
                                      tag=f"m{len(mask_tiles)}")
                    nc.sync.dma_start(
                        t[:, 0:m1 - m0],
                        mt_d.ap()[j * 128:(j + 1) * 128,
                                  qb * QB + m0:qb * QB + m1])
                    mask_tiles[key] = t
                return t

            # ---------------- attention stream ----------------
            class Stream:
                """One (b, h, qb): units alternate S/exp and d/PV."""

                def __init__(self, b, h, qb):
                    self.b, self.h, self.qb = b, h, qb
                    self.tb = b * S
                    ents = blocks[qb]
                    self.groups = [(ents[i:i + 2], i)
                                   for i in range(0, len(ents), 2)]
                    assert ents[0][1] == 0, "first entry must cover q0=0"
                    self.ne = len(ents)
                    self.qcol = self.tb + qb * QB
                    self.o_ps = None
                    self.es_acc = None
                    self.gi = 0
                    self.pends = []
                    self.done_units = False
                    self.inline_proj = False

                def unit_s(self, grp):
                    """S matmuls + exp + mask + denominator chain."""
                    h, tb = self.h, self.tb
                    q0g = grp[0][1]
                    n = len(grp)
                    sp = s_pools[sched['sc'] % len(s_pools)]
                    sched['sc'] += 1
                    s_t = sp.tile([128, 2 * QB], f32, tag="s",
                                  name="sps")
                    for idx, (j, q0, m0, m1) in enumerate(grp):
                        nc.tensor.matmul(
                            s_t[:, idx * QB + q0:(idx + 1) * QB],
                            kt_sb[h][:, tb + j * 128:tb + (j + 1) * 128],
                            qt_sb[h][:, self.qcol + q0:self.qcol + QB],
                            start=True, stop=True)
                    e_t = e_pool.tile([128, 2 * QB], bf16, tag="e",
                                      name="esb")
                    # one exp over [q0g, n*QB): start=True zeroed the
                    # whole bank, so unwritten score cols read exp(0)=1
                    # and are zeroed by the extended mask below
                    nc.scalar.activation(e_t[:, q0g:n * QB],
                                         s_t[:, q0g:n * QB], EXP,
                                         scale=scale)
                    for idx, (j, q0, m0, m1) in enumerate(grp):
                        hi = m1 if m1 is not None else \
                            (q0 if q0 > q0g else None)
                        if hi is not None and hi > q0g:
                            mt = mask_tile(j, self.qb, q0g, hi)
                            nc.vector.tensor_mul(
                                e_t[:, idx * QB + q0g:idx * QB + hi],
                                e_t[:, idx * QB + q0g:idx * QB + hi],
                                mt[:, 0:hi - q0g])
                    # denominator chain: es = e0 + e1, acc += es (f32)
                    if n == 2:
                        es_t = es_pool.tile([128, QB], bf16, tag="es",
                                            name="essb")
                        nc.vector.tensor_add(es_t[:, q0g:QB],
                                             e_t[:, q0g:QB],
                                             e_t[:, QB + q0g:2 * QB])
                        es_ap = es_t[:, q0g:QB]
                    else:
                        es_ap = e_t[:, q0g:QB]
                    if self.es_acc is None:
                        self.es_acc = acc_pool.tile([128, QB], f32r,
                                                    tag="acc", name="acc")
                        nc.vector.tensor_copy(self.es_acc[:, q0g:QB],
                                              es_ap)
                    else:
                        nc.vector.tensor_add(self.es_acc[:, q0g:QB],
                                             self.es_acc[:, q0g:QB],
                                             es_ap)
                    return (grp, e_t)

                def unit_pv(self, pend, g0):
                    """PV matmuls for a completed group."""
                    grp, e_t = pend
                    b, h = self.b, self.h
                    if self.o_ps is None:
                        self.o_ps = o_ps_pool.tile([128, QB], f32,
                                                   tag="o", name="ops")
                    for idx, (j, q0, m0, m1) in enumerate(grp):
                        gi = g0 + idx
                        nc.tensor.matmul(
                            self.o_ps[:, q0:QB],
                            v_sb[:, (b * NKT + j) * 256 + h * 128:
                                 (b * NKT + j) * 256 + (h + 1) * 128],
                            e_t[:, idx * QB + q0:(idx + 1) * QB],
                            start=(gi == 0), stop=(gi == self.ne - 1))

                def unit_tail(self):
                    if (self.h == HPC - 1
                            and (self.b, self.qb) == last_bqb):
                        # final stream: pipeline the tail per token-tile
                        # with inline projection (nothing else is left to
                        # hide the serial d->recip->mul chain behind)
                        db_ps = db_ps_pool.tile([128, QB], f32,
                                                tag="db", name="dbps")
                        rcp = rcp_pool.tile([128, QB], f32, tag="rcp",
                                            name="rcp")
                        for q4 in range(4):
                            sl = slice(q4 * 128, (q4 + 1) * 128)
                            nc.tensor.matmul(
                                db_ps[:, sl], ones_fr[:],
                                self.es_acc[:, sl],
                                start=q4 == 0, stop=q4 == 3)
                            nc.vector.reciprocal_approx_fast(
                                rcp[:, sl], db_ps[:, sl])
                            nc.vector.tensor_mul(
                                at_sb[self.h][:, self.qcol + q4 * 128:
                                              self.qcol + (q4 + 1) * 128],
                                self.o_ps[:, sl], rcp[:, sl])
                            for ch in range(4):
                                do_proj(self.b, self.qb * 4 + q4, ch)
                        self.inline_proj = True
                        return
                    db_ps = db_ps_pool.tile([128, QB], f32,
                                            tag="db", name="dbps")
                    nc.tensor.matmul(db_ps[:], ones_fr[:],
                                     self.es_acc[:], start=True,
                                     stop=True)
                    if dump and self.b == 0 and self.h == 0 and self.qb == 0:
                        dtmp = rcp_pool.tile([128, QB], f32, tag="dtmp",
                                             name="dtmp")
                        nc.vector.tensor_copy(dtmp[:], db_ps[:])
                        nc.sync.dma_start(d_dump.ap()[:, :], dtmp[:])
                    rcp = rcp_pool.tile([128, QB], f32, tag="rcp",
                                        name="rcp")
                    nc.vector.reciprocal_approx_fast(rcp[:], db_ps[:])
                    nc.vector.tensor_mul(
                        at_sb[self.h][:, self.qcol:self.qcol + QB],
                        self.o_ps[:], rcp[:])

                def step(self, s_only=False):
                    """Emit one unit. s_only: S/exp lookahead without PV
                    (safe while an older stream still owns the o bank)."""
                    if s_only:
                        if (self.gi < len(self.groups)
                                and len(self.pends) < 3):
                            grp, g0 = self.groups[self.gi]
                            self.pends.append((self.unit_s(grp), g0))
                            self.gi += 1
                            return True
                        return False
                    if self.gi < len(self.groups):
                        grp, g0 = self.groups[self.gi]
                        nxt = self.unit_s(grp)
                        if self.pends:
                            pend, pg = self.pends.pop(0)
                            self.unit_pv(pend, pg)
                        self.pends.append((nxt, g0))
                        self.gi += 1
                        return True
                    if self.pends:
                        pend, pg = self.pends.pop(0)
                        self.unit_pv(pend, pg)
                        return True
                    if not self.done_units:
                        self.unit_tail()
                        self.done_units = True
                        return True
                    return False

            # ---------------- projection units ----------------
            proj_queue = []   # (b, tt, ch)

            # chunk emission order interleaves batches so attention work
            # is available from the second chunk onward
            chunk_order = [0, 4, 1, 5, 2, 6, 3, 7]
            ysb_tiles = {}
            last_bqb = (chunk_order[-1] // 4, chunk_order[-1] % 4)

            def emit_proj_unit():
                do_proj(*proj_queue.pop(0))

            def do_proj(b, tt, ch):
                trow = b * S + tt * 128
                yp = y_pools[sched['yc'] % len(y_pools)]
                sched['yc'] += 1
                y_ps = yp.tile([128, 512], f32, tag="y", name="yps")
                for hh in range(HPC):
                    nc.tensor.matmul(
                        y_ps[:],
                        at_sb[hh][:, trow:trow + 128],
                        wo_sb[hh][:, ch * 512:(ch + 1) * 512],
                        start=(hh == 0), stop=(hh == HPC - 1))
                y_sb = ysb_tiles.get((b, tt))
                if y_sb is None:
                    y_sb = ysb_pool.tile([128, D], bf16, tag="ysb",
                                         name="ysb")
                    ysb_tiles[(b, tt)] = y_sb
                if (tt + ch) % 2 == 0:
                    nc.scalar.copy(y_sb[:, ch * 512:(ch + 1) * 512],
                                   y_ps[:])
                else:
                    nc.vector.tensor_copy(y_sb[:, ch * 512:(ch + 1) * 512],
                                          y_ps[:])
                last_qb = (b, tt // 4) == (last_bqb[0], last_bqb[1])
                if last_qb:
                    # final q-block: per-column-block DMA so the output
                    # transfers start before the kernel tail
                    nc.sync.dma_start(
                        y_d.ap()[trow:trow + 128,
                                 ch * 512:(ch + 1) * 512],
                        y_sb[:, ch * 512:(ch + 1) * 512])
                    if ch == 3:
                        del ysb_tiles[(b, tt)]
                elif ch == 3:
                    # all four column blocks done: one DMA per token row
                    nc.sync.dma_start(
                        y_d.ap()[trow:trow + 128, :], y_sb[:])
                    del ysb_tiles[(b, tt)]

            # ---------------- scheduler ----------------
            stream_list = []    # in ready order with chunk gates
            for qb in range(NQB):
                for b in range(B):
                    for h in range(HPC):
                        stream_list.append((4 * b + qb, Stream(b, h, qb)))
            sched = {'si': 0, 'tog': False, 'sc': 0, 'yc': 0, 'rr': 0}
            chunks_done = set()
            qk_done = set()     # chunk's q/k ready (before its v waves)

            def cur_stream():
                si = sched['si']
                if si >= len(stream_list):
                    return None
                gate, st = stream_list[si]
                if gate not in chunks_done:
                    return None
                return st

            def stream_done(st):
                return (st.done_units and not st.pends
                        and st.gi >= len(st.groups))

            def step_stream():
                # lead stream runs normally; the next stream may prefetch
                # S/exp units (no PV) so ACT stays fed during tails
                base = sched['si']
                rr = sched['rr']
                sched['rr'] = 1 - rr
                cands = [(base, False), (base + 1, True)]
                if rr:
                    cands.reverse()
                cands.append((base + 2, True))
                for si, s_only in cands:
                    if si >= len(stream_list):
                        continue
                    gate, st = stream_list[si]
                    if gate not in qk_done or stream_done(st):
                        continue
                    if gate not in chunks_done:
                        s_only = True   # v not ready: S/exp units only
                    if not st.step(s_only=s_only):
                        continue
                    while sched['si'] < len(stream_list):
                        g2, s2 = stream_list[sched['si']]
                        if not stream_done(s2):
                            break
                        if s2.h == HPC - 1 and not s2.inline_proj:
                            for t4 in range(4):
                                for ch in range(4):
                                    proj_queue.append(
                                        (s2.b, s2.qb * 4 + t4, ch))
                        sched['si'] += 1
                    return True
                return False

            def fill_slot():
                # alternate stream units and projection units; drain the
                # proj queue faster when it backs up
                tog = sched['tog']
                sched['tog'] = not tog
                if tog and proj_queue:
                    emit_proj_unit()
                    return True
                if step_stream():
                    return True
                if proj_queue:
                    emit_proj_unit()
                    return True
                return False

            # ---------------- phase 1 chunk waves + slots ----------------
            for pi, c in enumerate(chunk_order):
                nxt_c = chunk_order[pi + 1] if pi + 1 < NCH else None
                # qk waves: one e-tile each (0,1 = q_h0,q_h1; 2,3 = k_h0,k_h1)
                dsts = [qt_sb[0], qt_sb[1], kt_sb[0], kt_sb[1]]
                if pi == 0:
                    # first chunk: interleaved eighth DMAs so the first
                    # matmuls aren't waiting on one 2MB transfer
                    t = xt_pool.tile([128, NDT * 512], bf16, tag="xt")
                    xt_tiles[c] = t
                    w = NDT * 512 // 8
                    for i in range(8):
                        nc.sync.dma_start(
                            wqk_sb[:, i * w:(i + 1) * w],
                            wqk_d.ap()[:, i * w:(i + 1) * w])
                        nc.sync.dma_start(
                            t[:, i * w:(i + 1) * w],
                            xt_d.ap()[:, c * NDT * 512 + i * w:
                                      c * NDT * 512 + (i + 1) * w])
                    nc.sync.dma_start(wv_sb[:], wv_d.ap()[:, :])
                    for h in range(HPC):
                        nc.sync.dma_start(
                            wo_sb[h][:],
                            wo_d.ap()[h * 128:(h + 1) * 128, :])
                xt_t = xt_tile(c)
                for e in range(4):
                    qk_ps = qk_ps_pool.tile([128, 512], f32, tag="qk",
                                            name="qkps")
                    for kd in range(NDT):
                        if e == 2 and nxt_c is not None and kd == 0:
                            xt_tile(nxt_c)          # prefetch next chunk
                        nc.tensor.matmul(
                            qk_ps[:],
                            wqk_sb[:, kd * 512 + e * 128:
                                   kd * 512 + (e + 1) * 128],
                            xt_t[:, kd * 512:(kd + 1) * 512],
                            start=kd == 0, stop=kd == NDT - 1)
                        if kd % 4 == 1:
                            fill_slot()
                    nc.vector.tensor_copy(
                        dsts[e][:, c * 512:(c + 1) * 512], qk_ps[:])
                qk_done.add(c)
                # v waves: 2 token-tiles per wave, one PSUM bank each
                for wave in range(2):
                    v_ps = [v_ps_pool.tile([128, 256], f32, tag="v",
                                           name="vps")
                            for _ in range(2)]
                    for kd in range(NDT):
                        st, sp = kd == 0, kd == NDT - 1
                        for t2 in range(2):
                            tl = wave * 2 + t2
                            nc.tensor.matmul(
                                v_ps[t2][:],
                                xt_t[:, kd * 512 + tl * 128:
                                     kd * 512 + (tl + 1) * 128],
                                wv_sb[:, kd * 256:(kd + 1) * 256],
                                start=st, stop=sp)
                        if kd % 2 == 1:
                            fill_slot()
                    for t2 in range(2):
                        tok = c * 4 + wave * 2 + t2
                        nc.scalar.copy(
                            v_sb[:, tok * 256:(tok + 1) * 256], v_ps[t2][:])
                chunks_done.add(c)

            # ---------------- drain: attention + projection ----------------
            # phase-1 PSUM banks freed -> extra s/y pools for pipelining
            p1.close()
            s2_pool = stack.enter_context(
                tc.tile_pool(name="ps_s2", bufs=1, space="PSUM"))
            y2_pool = stack.enter_context(
                tc.tile_pool(name="ps_y2", bufs=1, space="PSUM"))
            s_pools.append(s2_pool)
            y_pools.append(y2_pool)
            while fill_slot():
                pass

            if dump:
                dsts = [qt_sb[0], qt_sb[1], kt_sb[0], kt_sb[1]]
                for e in range(4):
                    nc.sync.dma_start(
                        qk_dump.ap()[e * 128:(e + 1) * 128, :], dsts[e][:])
                nc.sync.dma_start(v_dump.ap()[:, :], v_sb[:])
                for h in range(HPC):
                    nc.sync.dma_start(
                        at_dump.ap()[h * 128:(h + 1) * 128, :], at_sb[h][:])

    nc.compile()
    return nc


last_results = None  # set when KERNEL_TRACE=1


def kernel(x, mask, W_qkv, W_o):
    import os
    import ml_dtypes
    from concourse.bass_utils import run_bass_kernel_spmd

    bf = ml_dtypes.bfloat16
    x = np.asarray(x, dtype=np.float32)
    mask_np = np.asarray(mask).astype(bool)
    W_qkv = np.asarray(W_qkv, dtype=np.float32)
    W_o = np.asarray(W_o, dtype=np.float32)
    m2 = np.broadcast_to(mask_np, (1, 1, S, S))[0, 0]

    key = m2.tobytes()
    nc = _cache.get(key)
    if nc is None:
        nc = _build(m2)
        _cache[key] = nc

    # xt packed: [p, (c*16+kd)*512 + t] = x^T[kd*128+p, c*512+t]
    xt_full = x.reshape(T, D).T                                  # [D, T]
    xtp = np.ascontiguousarray(
        xt_full.reshape(NDT, 128, NCH, 512).transpose(1, 2, 0, 3)
        .reshape(128, NCH * NDT * 512)).astype(bf)
    mt = np.ascontiguousarray(m2.T.astype(np.float32)).astype(bf)

    in_maps = []
    for c in range(NCORES):
        hA, hB = HPC * c, HPC * c + 1
        q_rows = list(range(hA * DK, (hA + 1) * DK)) + \
                 list(range(hB * DK, (hB + 1) * DK))
        k_rows = [D + r for r in q_rows]
        v_rows = [2 * D + r for r in q_rows]
        wqk = W_qkv[q_rows + k_rows, :].T                        # [D, 512]
        # packed: [p, kd*512 + col] = wqk[kd*128+p, col]
        wqkp = np.ascontiguousarray(
            wqk.reshape(NDT, 128, 512).transpose(1, 0, 2)
            .reshape(128, NDT * 512)).astype(bf)
        wv = W_qkv[v_rows, :].T                                  # [D, 256]
        wvp = np.ascontiguousarray(
            wv.reshape(NDT, 128, 256).transpose(1, 0, 2)
            .reshape(128, NDT * 256)).astype(bf)
        wo = np.ascontiguousarray(W_o[:, q_rows].T).astype(bf)
        in_maps.append({"xt": xtp, "wqk": wqkp, "wv": wvp, "wo": wo,
                        "mt": mt})

    trace = bool(os.environ.get("KERNEL_TRACE"))
    res = run_bass_kernel_spmd(nc, in_maps, core_ids=list(range(NCORES)),
                               trace=trace)
    if trace:
        global last_results
        last_results = res
    y = res.results[0]["y"].astype(np.float32)
    for c in range(1, NCORES):
        y += res.results[c]["y"].astype(np.float32)
    return y.reshape(B, S, D)
